# revision 3
# baseline (speedup 1.0000x reference)
"""Trainium2 Bass kernel for the ChessNN (NNUE-style) model.

Computation:
    w   = white @ W0.T + b0                  # [B, 256]
    b   = black @ W0.T + b0                  # [B, 256]
    acc = stm * [w, b] + (1-stm) * [b, w]    # [B, 512]
    l1  = clip(acc, 0, 1)
    l2  = clip(l1 @ W1.T + b1, 0, 1)         # [B, 32]
    l3  = clip(l2 @ W2.T + b2, 0, 1)         # [B, 32]
    y   = l3 @ W3.T + b3                     # [B, 1]

Strategy: data-parallel over the batch across 8 NeuronCores (512 rows each).
The l0 GEMM streams X and W0 feature-major (host-transposed, lossless) so the
PE contracts over the partition dim with no on-chip transposes; inputs are
cast f32->bf16 during the SWDGE DMA (f32 accumulation in PSUM), and the whole
epilogue runs in the transposed orientation [features, batch] where every bias
is a per-partition scalar and the final [1, 512] result stores contiguously.
"""

import numpy as np

NCORES = 8
B = 4096
BL = B // NCORES  # 512 batch rows per core
F = 41024
P = 128
KT = 321  # ceil(F / P) k-tiles
FP = KT * P  # 41088, feature dim zero-padded
M = 256  # l0 output width
GROUP = 8  # k-tiles per DMA chunk

# bf16 multiplies with f32 accumulation for the big GEMM (4x faster PE than
# fp32's 2-pass mode); the tiny MLP stays exact f32.
GEMM_BF16 = True

_cache = {}


def _build_program():
    import concourse.mybir as mybir
    from concourse import bacc
    from concourse.tile import TileContext

    f32 = mybir.dt.float32
    gdt = mybir.dt.bfloat16 if GEMM_BF16 else f32
    Alu = mybir.AluOpType

    nc = bacc.Bacc("TRN2", target_bir_lowering=False, debug=False,
                   enable_asserts=False)

    xw = nc.dram_tensor("xw_t", [FP, BL], f32, kind="ExternalInput").ap()
    xb = nc.dram_tensor("xb_t", [FP, BL], f32, kind="ExternalInput").ap()
    w0 = nc.dram_tensor("w0_t", [FP, M], f32, kind="ExternalInput").ap()
    stm = nc.dram_tensor("stm_bc", [P, BL], f32, kind="ExternalInput").ap()
    b0 = nc.dram_tensor("b0_r", [P, 2], f32, kind="ExternalInput").ap()
    w1 = nc.dram_tensor("w1_t", [2 * M, 32], f32, kind="ExternalInput").ap()
    b1 = nc.dram_tensor("b1_r", [32, 1], f32, kind="ExternalInput").ap()
    w2 = nc.dram_tensor("w2_t", [32, 32], f32, kind="ExternalInput").ap()
    b2 = nc.dram_tensor("b2_r", [32, 1], f32, kind="ExternalInput").ap()
    w3 = nc.dram_tensor("w3_t", [32, 1], f32, kind="ExternalInput").ap()
    b3 = nc.dram_tensor("b3_r", [1, 1], f32, kind="ExternalInput").ap()
    y = nc.dram_tensor("y", [1, BL], f32, kind="ExternalOutput").ap()

    with TileContext(nc) as tc:
        with (
            tc.tile_pool(name="xpool", bufs=4) as xpool,
            tc.tile_pool(name="wpool", bufs=4) as wpool,
            tc.tile_pool(name="cpool", bufs=1) as cpool,
            tc.tile_pool(name="epi", bufs=1) as epi,
            tc.tile_pool(name="psum", bufs=1, space="PSUM") as psum,
        ):
            # ---- constants (loaded once) ----
            stm_t = cpool.tile([P, BL], f32, tag="stm")
            nc.sync.dma_start(out=stm_t[:], in_=stm)
            b0_t = cpool.tile([P, 2], f32, tag="b0")
            nc.sync.dma_start(out=b0_t[:], in_=b0)
            w1_t = cpool.tile([P, 4, 32], f32, tag="w1")
            nc.sync.dma_start(out=w1_t[:], in_=w1.rearrange("(g p) m -> p g m", p=P))
            b1_t = cpool.tile([32, 1], f32, tag="b1")
            nc.sync.dma_start(out=b1_t[:], in_=b1)
            w2_t = cpool.tile([32, 32], f32, tag="w2")
            nc.sync.dma_start(out=w2_t[:], in_=w2)
            b2_t = cpool.tile([32, 1], f32, tag="b2")
            nc.sync.dma_start(out=b2_t[:], in_=b2)
            w3_t = cpool.tile([32, 1], f32, tag="w3")
            nc.sync.dma_start(out=w3_t[:], in_=w3)
            b3_t = cpool.tile([1, 1], f32, tag="b3")
            nc.sync.dma_start(out=b3_t[:], in_=b3)

            # ---- l0 GEMM: accT[f', b] accumulated over 321 k-tiles ----
            # acc[0]=white h0, acc[1]=white h1, acc[2]=black h0, acc[3]=black h1
            acc = [psum.tile([P, BL], f32, tag=f"acc{i}", name=f"acc{i}") for i in range(4)]

            k0 = 0
            groups = [GROUP] * (KT // GROUP) + ([KT % GROUP] if KT % GROUP else [])
            for gsz in groups:
                xw_t = xpool.tile([P, gsz, BL], gdt, tag="xw")
                xb_t = xpool.tile([P, gsz, BL], gdt, tag="xb")
                w0_t = wpool.tile([P, gsz, M], gdt, tag="w0")
                rows = slice(k0 * P, (k0 + gsz) * P)
                dma = nc.gpsimd if GEMM_BF16 else nc.sync
                dma.dma_start(out=xw_t[:], in_=xw[rows, :].rearrange("(g p) b -> p g b", p=P))
                dma.dma_start(out=xb_t[:], in_=xb[rows, :].rearrange("(g p) b -> p g b", p=P))
                dma.dma_start(out=w0_t[:], in_=w0[rows, :].rearrange("(g p) m -> p g m", p=P))
                for g in range(gsz):
                    k = k0 + g
                    st, sp = (k == 0), (k == KT - 1)
                    nc.tensor.matmul(acc[0], w0_t[:, g, 0:P], xw_t[:, g, :], start=st, stop=sp)
                    nc.tensor.matmul(acc[1], w0_t[:, g, P:M], xw_t[:, g, :], start=st, stop=sp)
                    nc.tensor.matmul(acc[2], w0_t[:, g, 0:P], xb_t[:, g, :], start=st, stop=sp)
                    nc.tensor.matmul(acc[3], w0_t[:, g, P:M], xb_t[:, g, :], start=st, stop=sp)
                k0 += gsz

            # ---- blend + bias + clip -> l1T [512 f', 512 b] ----
            # acc col f' < 256:  stm*w + (1-stm)*b = b + stm*(w-b)
            # acc col f' >= 256: stm*b + (1-stm)*w = w - stm*(w-b)
            l1 = epi.tile([P, 4, BL], f32, tag="l1")
            for h in range(2):
                w_ps, b_ps = acc[h], acc[2 + h]
                # HW: an op may read at most one non-scalar input from PSUM.
                w_sb = epi.tile([P, BL], f32, tag=f"wsb{h}")
                nc.vector.tensor_copy(out=w_sb[:], in_=w_ps[:])
                t = epi.tile([P, BL], f32, tag=f"t{h}")
                nc.vector.tensor_tensor(t[:], w_sb[:], b_ps[:], Alu.subtract)
                nc.vector.tensor_tensor(t[:], t[:], stm_t[:], Alu.mult)
                bias = b0_t[:, h : h + 1]
                # l1[f' = h*128 .. ]: b + t
                nc.vector.tensor_tensor(l1[:, h, :], b_ps[:], t[:], Alu.add)
                nc.vector.tensor_scalar(l1[:, h, :], l1[:, h, :], bias, 0.0, Alu.add, Alu.max)
                nc.vector.tensor_scalar_min(l1[:, h, :], l1[:, h, :], 1.0)
                # l1[f' = 256 + h*128 .. ]: w - t
                nc.vector.tensor_tensor(l1[:, 2 + h, :], w_sb[:], t[:], Alu.subtract)
                nc.vector.tensor_scalar(l1[:, 2 + h, :], l1[:, 2 + h, :], bias, 0.0, Alu.add, Alu.max)
                nc.vector.tensor_scalar_min(l1[:, 2 + h, :], l1[:, 2 + h, :], 1.0)

            # ---- MLP (exact f32; transposed orientation) ----
            l2ps = psum.tile([32, BL], f32, tag="l2ps")
            for g in range(4):
                nc.tensor.matmul(l2ps, w1_t[:, g, :], l1[:, g, :], start=(g == 0), stop=(g == 3))
            l2 = epi.tile([32, BL], f32, tag="l2")
            nc.vector.tensor_scalar(l2[:], l2ps[:], b1_t[:], 0.0, Alu.add, Alu.max)
            nc.vector.tensor_scalar_min(l2[:], l2[:], 1.0)

            l3ps = psum.tile([32, BL], f32, tag="l3ps")
            nc.tensor.matmul(l3ps, w2_t[:], l2[:], start=True, stop=True)
            l3 = epi.tile([32, BL], f32, tag="l3")
            nc.vector.tensor_scalar(l3[:], l3ps[:], b2_t[:], 0.0, Alu.add, Alu.max)
            nc.vector.tensor_scalar_min(l3[:], l3[:], 1.0)

            yps = psum.tile([1, BL], f32, tag="yps")
            nc.tensor.matmul(yps, w3_t[:], l3[:], start=True, stop=True)
            y_t = epi.tile([1, BL], f32, tag="y")
            nc.vector.tensor_scalar_add(y_t[:], yps[:], b3_t[:])
            nc.sync.dma_start(out=y, in_=y_t[:])

    nc.compile()
    return nc


def _prep_inputs(white_features, black_features, stm, W0, b0, W1, b1, W2, b2, W3, b3):
    """Host-side (lossless) relayout + batch sharding."""
    wf = np.asarray(white_features, dtype=np.float32)
    bf = np.asarray(black_features, dtype=np.float32)
    stm = np.asarray(stm, dtype=np.float32).reshape(B)
    W0 = np.asarray(W0, dtype=np.float32)
    b0 = np.asarray(b0, dtype=np.float32)
    W1 = np.asarray(W1, dtype=np.float32)
    b1 = np.asarray(b1, dtype=np.float32)
    W2 = np.asarray(W2, dtype=np.float32)
    b2 = np.asarray(b2, dtype=np.float32)
    W3 = np.asarray(W3, dtype=np.float32)
    b3 = np.asarray(b3, dtype=np.float32)

    w0_t = np.zeros((FP, M), np.float32)
    w0_t[:F] = W0.T
    b0_r = np.ascontiguousarray(b0.reshape(2, P).T)  # [128, 2]
    w1_t = np.ascontiguousarray(W1.T)  # [512, 32]
    b1_r = b1.reshape(32, 1)
    w2_t = np.ascontiguousarray(W2.T)  # [32, 32]
    b2_r = b2.reshape(32, 1)
    w3_t = np.ascontiguousarray(W3.T)  # [32, 1]
    b3_r = b3.reshape(1, 1)

    in_maps = []
    for c in range(NCORES):
        rows = slice(c * BL, (c + 1) * BL)
        xw_t = np.zeros((FP, BL), np.float32)
        xw_t[:F] = wf[rows].T
        xb_t = np.zeros((FP, BL), np.float32)
        xb_t[:F] = bf[rows].T
        stm_bc = np.ascontiguousarray(
            np.broadcast_to(stm[rows][None, :], (P, BL)), dtype=np.float32)
        in_maps.append({
            "xw_t": xw_t, "xb_t": xb_t, "w0_t": w0_t, "stm_bc": stm_bc,
            "b0_r": b0_r, "w1_t": w1_t, "b1_r": b1_r, "w2_t": w2_t,
            "b2_r": b2_r, "w3_t": w3_t, "b3_r": b3_r,
        })
    return in_maps


def _run(in_maps, trace=False, **kw):
    from concourse.bass_utils import run_bass_kernel_spmd

    if "nc" not in _cache:
        _cache["nc"] = _build_program()
    return run_bass_kernel_spmd(
        _cache["nc"], in_maps, core_ids=list(range(NCORES)), trace=trace, **kw)


def kernel(**inputs) -> np.ndarray:
    in_maps = _prep_inputs(**inputs)
    res = _run(in_maps, trace=False)
    out = np.empty((B, 1), np.float32)
    for c in range(NCORES):
        out[c * BL : (c + 1) * BL, 0] = res.results[c]["y"].reshape(BL)
    return out


# revision 7
# speedup vs baseline: 1.0033x; 1.0033x over previous
"""Trainium2 Bass kernel for the ChessNN (NNUE-style) model.

Computation:
    w   = white @ W0.T + b0                  # [B, 256]
    b   = black @ W0.T + b0                  # [B, 256]
    acc = stm * [w, b] + (1-stm) * [b, w]    # [B, 512]
    l1  = clip(acc, 0, 1)
    l2  = clip(l1 @ W1.T + b1, 0, 1)         # [B, 32]
    l3  = clip(l2 @ W2.T + b2, 0, 1)         # [B, 32]
    y   = l3 @ W3.T + b3                     # [B, 1]

Strategy: data-parallel over the batch across 8 NeuronCores (512 rows each).
The l0 GEMM streams X and W0 feature-major (host-transposed, lossless) so the
PE contracts over the partition dim with no on-chip transposes; inputs are
cast f32->bf16 during the SWDGE DMA (f32 accumulation in PSUM), and the whole
epilogue runs in the transposed orientation [features, batch] where every bias
is a per-partition scalar and the final [1, 512] result stores contiguously.
"""

import numpy as np

NCORES = 8
B = 4096
BL = B // NCORES  # 512 batch rows per core
F = 41024
P = 128
KT = 321  # ceil(F / P) k-tiles
FP = KT * P  # 41088, feature dim zero-padded
M = 256  # l0 output width
GROUP = 16  # steady-state k-tiles per DMA chunk
RAMP = [4, 4, 8]  # small leading chunks so the PE starts early

# bf16 multiplies with f32 accumulation for the big GEMM (4x faster PE than
# fp32's 2-pass mode); the tiny MLP stays exact f32.
GEMM_BF16 = True

_cache = {}


def _build_program():
    import concourse.mybir as mybir
    from concourse import bacc
    from concourse.tile import TileContext

    f32 = mybir.dt.float32
    gdt = mybir.dt.bfloat16 if GEMM_BF16 else f32
    Alu = mybir.AluOpType

    nc = bacc.Bacc("TRN2", target_bir_lowering=False, debug=False,
                   enable_asserts=False)

    xw = nc.dram_tensor("xw_t", [FP, BL], f32, kind="ExternalInput").ap()
    xb = nc.dram_tensor("xb_t", [FP, BL], f32, kind="ExternalInput").ap()
    w0 = nc.dram_tensor("w0_t", [FP, M], f32, kind="ExternalInput").ap()
    stm = nc.dram_tensor("stm_bc", [P, BL], f32, kind="ExternalInput").ap()
    b0 = nc.dram_tensor("b0_r", [P, 2], f32, kind="ExternalInput").ap()
    w1 = nc.dram_tensor("w1_t", [2 * M, 32], f32, kind="ExternalInput").ap()
    b1 = nc.dram_tensor("b1_r", [32, 1], f32, kind="ExternalInput").ap()
    w2 = nc.dram_tensor("w2_t", [32, 32], f32, kind="ExternalInput").ap()
    b2 = nc.dram_tensor("b2_r", [32, 1], f32, kind="ExternalInput").ap()
    w3 = nc.dram_tensor("w3_t", [32, 1], f32, kind="ExternalInput").ap()
    b3 = nc.dram_tensor("b3_r", [1, 1], f32, kind="ExternalInput").ap()
    y = nc.dram_tensor("y", [1, BL], f32, kind="ExternalOutput").ap()

    with TileContext(nc) as tc:
        with (
            tc.tile_pool(name="xpool", bufs=3) as xpool,
            tc.tile_pool(name="wpool", bufs=3) as wpool,
            tc.tile_pool(name="cpool", bufs=1) as cpool,
            tc.tile_pool(name="epi", bufs=1) as epi,
            tc.tile_pool(name="psum", bufs=1, space="PSUM") as psum,
        ):
            # ---- constants (loaded once) ----
            stm_t = cpool.tile([P, BL], f32, tag="stm")
            nc.sync.dma_start(out=stm_t[:], in_=stm)
            b0_t = cpool.tile([P, 2], f32, tag="b0")
            nc.sync.dma_start(out=b0_t[:], in_=b0)
            w1_t = cpool.tile([P, 4, 32], f32, tag="w1")
            nc.sync.dma_start(out=w1_t[:], in_=w1.rearrange("(g p) m -> p g m", p=P))
            b1_t = cpool.tile([32, 1], f32, tag="b1")
            nc.sync.dma_start(out=b1_t[:], in_=b1)
            w2_t = cpool.tile([32, 32], f32, tag="w2")
            nc.sync.dma_start(out=w2_t[:], in_=w2)
            b2_t = cpool.tile([32, 1], f32, tag="b2")
            nc.sync.dma_start(out=b2_t[:], in_=b2)
            w3_t = cpool.tile([32, 1], f32, tag="w3")
            nc.sync.dma_start(out=w3_t[:], in_=w3)
            b3_t = cpool.tile([1, 1], f32, tag="b3")
            nc.sync.dma_start(out=b3_t[:], in_=b3)

            # ---- l0 GEMM: accT[f', b] accumulated over 321 k-tiles ----
            # acc[0]=white h0, acc[1]=white h1, acc[2]=black h0, acc[3]=black h1
            acc = [psum.tile([P, BL], f32, tag=f"acc{i}", name=f"acc{i}") for i in range(4)]

            k0 = 0
            rest = KT - sum(RAMP)
            groups = RAMP + [GROUP] * (rest // GROUP) + ([rest % GROUP] if rest % GROUP else [])
            for gsz in groups:
                xw_t = xpool.tile([P, gsz, BL], gdt, tag="xw")
                xb_t = xpool.tile([P, gsz, BL], gdt, tag="xb")
                w0_t = wpool.tile([P, gsz, M], gdt, tag="w0")
                rows = slice(k0 * P, (k0 + gsz) * P)
                dma = nc.gpsimd if GEMM_BF16 else nc.sync
                dma.dma_start(out=xw_t[:], in_=xw[rows, :].rearrange("(g p) b -> p g b", p=P))
                dma.dma_start(out=xb_t[:], in_=xb[rows, :].rearrange("(g p) b -> p g b", p=P))
                dma.dma_start(out=w0_t[:], in_=w0[rows, :].rearrange("(g p) m -> p g m", p=P))
                for g in range(gsz):
                    k = k0 + g
                    st, sp = (k == 0), (k == KT - 1)
                    # consecutive matmuls share the stationary operand
                    nc.tensor.matmul(acc[0], w0_t[:, g, 0:P], xw_t[:, g, :], start=st, stop=sp)
                    nc.tensor.matmul(acc[2], w0_t[:, g, 0:P], xb_t[:, g, :], start=st, stop=sp)
                    nc.tensor.matmul(acc[1], w0_t[:, g, P:M], xw_t[:, g, :], start=st, stop=sp)
                    nc.tensor.matmul(acc[3], w0_t[:, g, P:M], xb_t[:, g, :], start=st, stop=sp)
                k0 += gsz

            # ---- blend + bias + clip -> l1T [512 f', 512 b] ----
            # acc col f' < 256:  stm*w + (1-stm)*b = b + stm*(w-b)
            # acc col f' >= 256: stm*b + (1-stm)*w = w - stm*(w-b)
            l1 = epi.tile([P, 4, BL], f32, tag="l1")
            for h in range(2):
                w_ps, b_ps = acc[h], acc[2 + h]
                # HW: an op may read at most one non-scalar input from PSUM.
                w_sb = epi.tile([P, BL], f32, tag=f"wsb{h}")
                nc.vector.tensor_copy(out=w_sb[:], in_=w_ps[:])
                t = epi.tile([P, BL], f32, tag=f"t{h}")
                nc.vector.tensor_tensor(t[:], w_sb[:], b_ps[:], Alu.subtract)
                nc.vector.tensor_tensor(t[:], t[:], stm_t[:], Alu.mult)
                bias = b0_t[:, h : h + 1]
                # l1[f' = h*128 .. ]: b + t
                nc.vector.tensor_tensor(l1[:, h, :], b_ps[:], t[:], Alu.add)
                nc.vector.tensor_scalar(l1[:, h, :], l1[:, h, :], bias, 0.0, Alu.add, Alu.max)
                nc.vector.tensor_scalar_min(l1[:, h, :], l1[:, h, :], 1.0)
                # l1[f' = 256 + h*128 .. ]: w - t
                nc.vector.tensor_tensor(l1[:, 2 + h, :], w_sb[:], t[:], Alu.subtract)
                nc.vector.tensor_scalar(l1[:, 2 + h, :], l1[:, 2 + h, :], bias, 0.0, Alu.add, Alu.max)
                nc.vector.tensor_scalar_min(l1[:, 2 + h, :], l1[:, 2 + h, :], 1.0)

            # ---- MLP (exact f32; transposed orientation) ----
            l2ps = psum.tile([32, BL], f32, tag="l2ps")
            for g in range(4):
                nc.tensor.matmul(l2ps, w1_t[:, g, :], l1[:, g, :], start=(g == 0), stop=(g == 3))
            l2 = epi.tile([32, BL], f32, tag="l2")
            nc.vector.tensor_scalar(l2[:], l2ps[:], b1_t[:], 0.0, Alu.add, Alu.max)
            nc.vector.tensor_scalar_min(l2[:], l2[:], 1.0)

            l3ps = psum.tile([32, BL], f32, tag="l3ps")
            nc.tensor.matmul(l3ps, w2_t[:], l2[:], start=True, stop=True)
            l3 = epi.tile([32, BL], f32, tag="l3")
            nc.vector.tensor_scalar(l3[:], l3ps[:], b2_t[:], 0.0, Alu.add, Alu.max)
            nc.vector.tensor_scalar_min(l3[:], l3[:], 1.0)

            yps = psum.tile([1, BL], f32, tag="yps")
            nc.tensor.matmul(yps, w3_t[:], l3[:], start=True, stop=True)
            y_t = epi.tile([1, BL], f32, tag="y")
            nc.vector.tensor_scalar_add(y_t[:], yps[:], b3_t[:])
            nc.sync.dma_start(out=y, in_=y_t[:])

    nc.compile()
    return nc


def _prep_inputs(white_features, black_features, stm, W0, b0, W1, b1, W2, b2, W3, b3):
    """Host-side (lossless) relayout + batch sharding."""
    wf = np.asarray(white_features, dtype=np.float32)
    bf = np.asarray(black_features, dtype=np.float32)
    stm = np.asarray(stm, dtype=np.float32).reshape(B)
    W0 = np.asarray(W0, dtype=np.float32)
    b0 = np.asarray(b0, dtype=np.float32)
    W1 = np.asarray(W1, dtype=np.float32)
    b1 = np.asarray(b1, dtype=np.float32)
    W2 = np.asarray(W2, dtype=np.float32)
    b2 = np.asarray(b2, dtype=np.float32)
    W3 = np.asarray(W3, dtype=np.float32)
    b3 = np.asarray(b3, dtype=np.float32)

    w0_t = np.zeros((FP, M), np.float32)
    w0_t[:F] = W0.T
    b0_r = np.ascontiguousarray(b0.reshape(2, P).T)  # [128, 2]
    w1_t = np.ascontiguousarray(W1.T)  # [512, 32]
    b1_r = b1.reshape(32, 1)
    w2_t = np.ascontiguousarray(W2.T)  # [32, 32]
    b2_r = b2.reshape(32, 1)
    w3_t = np.ascontiguousarray(W3.T)  # [32, 1]
    b3_r = b3.reshape(1, 1)

    in_maps = []
    for c in range(NCORES):
        rows = slice(c * BL, (c + 1) * BL)
        xw_t = np.zeros((FP, BL), np.float32)
        xw_t[:F] = wf[rows].T
        xb_t = np.zeros((FP, BL), np.float32)
        xb_t[:F] = bf[rows].T
        stm_bc = np.ascontiguousarray(
            np.broadcast_to(stm[rows][None, :], (P, BL)), dtype=np.float32)
        in_maps.append({
            "xw_t": xw_t, "xb_t": xb_t, "w0_t": w0_t, "stm_bc": stm_bc,
            "b0_r": b0_r, "w1_t": w1_t, "b1_r": b1_r, "w2_t": w2_t,
            "b2_r": b2_r, "w3_t": w3_t, "b3_r": b3_r,
        })
    return in_maps


def _run(in_maps, trace=False, **kw):
    from concourse.bass_utils import run_bass_kernel_spmd

    if "nc" not in _cache:
        _cache["nc"] = _build_program()
    return run_bass_kernel_spmd(
        _cache["nc"], in_maps, core_ids=list(range(NCORES)), trace=trace, **kw)


def kernel(**inputs) -> np.ndarray:
    in_maps = _prep_inputs(**inputs)
    res = _run(in_maps, trace=False)
    out = np.empty((B, 1), np.float32)
    for c in range(NCORES):
        out[c * BL : (c + 1) * BL, 0] = res.results[c]["y"].reshape(BL)
    return out


# revision 8
# speedup vs baseline: 1.1975x; 1.1935x over previous
"""Trainium2 Bass kernel for the ChessNN (NNUE-style) model.

Computation:
    w   = white @ W0.T + b0                  # [B, 256]
    b   = black @ W0.T + b0                  # [B, 256]
    acc = stm * [w, b] + (1-stm) * [b, w]    # [B, 512]
    l1  = clip(acc, 0, 1)
    l2  = clip(l1 @ W1.T + b1, 0, 1)         # [B, 32]
    l3  = clip(l2 @ W2.T + b2, 0, 1)         # [B, 32]
    y   = l3 @ W3.T + b3                     # [B, 1]

Strategy: data-parallel over the batch across 8 NeuronCores (512 rows each).
The l0 GEMM streams X and W0 feature-major (host-transposed, lossless) so the
PE contracts over the partition dim with no on-chip transposes; inputs are
cast f32->bf16 during the SWDGE DMA (f32 accumulation in PSUM), and the whole
epilogue runs in the transposed orientation [features, batch] where every bias
is a per-partition scalar and the final [1, 512] result stores contiguously.
"""

import numpy as np

NCORES = 8
B = 4096
BL = B // NCORES  # 512 batch rows per core
F = 41024
P = 128
KT = 321  # ceil(F / P) k-tiles
FP = KT * P  # 41088, feature dim zero-padded
M = 256  # l0 output width
GROUP = 12  # steady-state k-tiles per DMA chunk
RAMP = [2, 2, 4, 8]  # small leading chunks so the PE starts early

# bf16 multiplies with f32 accumulation for the big GEMM (4x faster PE than
# fp32's 2-pass mode); the tiny MLP stays exact f32.
GEMM_BF16 = True

_cache = {}


def _build_program():
    import concourse.mybir as mybir
    from concourse import bacc
    from concourse.tile import TileContext

    f32 = mybir.dt.float32
    gdt = mybir.dt.bfloat16 if GEMM_BF16 else f32
    Alu = mybir.AluOpType

    nc = bacc.Bacc("TRN2", target_bir_lowering=False, debug=False,
                   enable_asserts=False)

    xw = nc.dram_tensor("xw_t", [FP, BL], f32, kind="ExternalInput").ap()
    xb = nc.dram_tensor("xb_t", [FP, BL], f32, kind="ExternalInput").ap()
    w0 = nc.dram_tensor("w0_t", [FP, M], f32, kind="ExternalInput").ap()
    stm = nc.dram_tensor("stm_bc", [P, BL], f32, kind="ExternalInput").ap()
    b0 = nc.dram_tensor("b0_r", [P, 2], f32, kind="ExternalInput").ap()
    w1 = nc.dram_tensor("w1_t", [2 * M, 32], f32, kind="ExternalInput").ap()
    b1 = nc.dram_tensor("b1_r", [32, 1], f32, kind="ExternalInput").ap()
    w2 = nc.dram_tensor("w2_t", [32, 32], f32, kind="ExternalInput").ap()
    b2 = nc.dram_tensor("b2_r", [32, 1], f32, kind="ExternalInput").ap()
    w3 = nc.dram_tensor("w3_t", [32, 1], f32, kind="ExternalInput").ap()
    b3 = nc.dram_tensor("b3_r", [1, 1], f32, kind="ExternalInput").ap()
    y = nc.dram_tensor("y", [1, BL], f32, kind="ExternalOutput").ap()

    with TileContext(nc) as tc:
        with (
            tc.tile_pool(name="xpool", bufs=4) as xpool,
            tc.tile_pool(name="wpool", bufs=4) as wpool,
            tc.tile_pool(name="cpool", bufs=1) as cpool,
            tc.tile_pool(name="epi", bufs=1) as epi,
            tc.tile_pool(name="psum", bufs=1, space="PSUM") as psum,
        ):
            # ---- constants (loaded once) ----
            stm_t = cpool.tile([P, BL], f32, tag="stm")
            nc.sync.dma_start(out=stm_t[:], in_=stm)
            b0_t = cpool.tile([P, 2], f32, tag="b0")
            nc.sync.dma_start(out=b0_t[:], in_=b0)
            w1_t = cpool.tile([P, 4, 32], f32, tag="w1")
            nc.sync.dma_start(out=w1_t[:], in_=w1.rearrange("(g p) m -> p g m", p=P))
            b1_t = cpool.tile([32, 1], f32, tag="b1")
            nc.sync.dma_start(out=b1_t[:], in_=b1)
            w2_t = cpool.tile([32, 32], f32, tag="w2")
            nc.sync.dma_start(out=w2_t[:], in_=w2)
            b2_t = cpool.tile([32, 1], f32, tag="b2")
            nc.sync.dma_start(out=b2_t[:], in_=b2)
            w3_t = cpool.tile([32, 1], f32, tag="w3")
            nc.sync.dma_start(out=w3_t[:], in_=w3)
            b3_t = cpool.tile([1, 1], f32, tag="b3")
            nc.sync.dma_start(out=b3_t[:], in_=b3)

            # ---- l0 GEMM: accT[f', b] accumulated over 321 k-tiles ----
            # acc[0]=white h0, acc[1]=white h1, acc[2]=black h0, acc[3]=black h1
            acc = [psum.tile([P, BL], f32, tag=f"acc{i}", name=f"acc{i}") for i in range(4)]

            k0 = 0
            rest = KT - sum(RAMP)
            groups = RAMP + [GROUP] * (rest // GROUP) + ([rest % GROUP] if rest % GROUP else [])
            for gsz in groups:
                xw_t = xpool.tile([P, gsz, BL], gdt, tag="xw")
                xb_t = xpool.tile([P, gsz, BL], gdt, tag="xb")
                w0_t = wpool.tile([P, gsz, M], gdt, tag="w0")
                rows = slice(k0 * P, (k0 + gsz) * P)
                dma = nc.gpsimd if GEMM_BF16 else nc.sync
                dma.dma_start(out=w0_t[:], in_=w0[rows, :].rearrange("(g p) m -> p g m", p=P))
                dma.dma_start(out=xw_t[:], in_=xw[rows, :].rearrange("(g p) b -> p g b", p=P))
                dma.dma_start(out=xb_t[:], in_=xb[rows, :].rearrange("(g p) b -> p g b", p=P))
                for g in range(gsz):
                    k = k0 + g
                    st, sp = (k == 0), (k == KT - 1)
                    # consecutive matmuls share the stationary operand
                    nc.tensor.matmul(acc[0], w0_t[:, g, 0:P], xw_t[:, g, :], start=st, stop=sp)
                    nc.tensor.matmul(acc[2], w0_t[:, g, 0:P], xb_t[:, g, :], start=st, stop=sp)
                    nc.tensor.matmul(acc[1], w0_t[:, g, P:M], xw_t[:, g, :], start=st, stop=sp)
                    nc.tensor.matmul(acc[3], w0_t[:, g, P:M], xb_t[:, g, :], start=st, stop=sp)
                k0 += gsz

            # ---- blend + bias + clip -> l1T [512 f', 512 b] ----
            # acc col f' < 256:  stm*w + (1-stm)*b = b + stm*(w-b)
            # acc col f' >= 256: stm*b + (1-stm)*w = w - stm*(w-b)
            l1 = epi.tile([P, 4, BL], f32, tag="l1")
            for h in range(2):
                w_ps, b_ps = acc[h], acc[2 + h]
                # HW: an op may read at most one non-scalar input from PSUM.
                w_sb = epi.tile([P, BL], f32, tag=f"wsb{h}")
                nc.vector.tensor_copy(out=w_sb[:], in_=w_ps[:])
                t = epi.tile([P, BL], f32, tag=f"t{h}")
                nc.vector.tensor_tensor(t[:], w_sb[:], b_ps[:], Alu.subtract)
                nc.vector.tensor_tensor(t[:], t[:], stm_t[:], Alu.mult)
                bias = b0_t[:, h : h + 1]
                # l1[f' = h*128 .. ]: b + t
                nc.vector.tensor_tensor(l1[:, h, :], b_ps[:], t[:], Alu.add)
                nc.vector.tensor_scalar(l1[:, h, :], l1[:, h, :], bias, 0.0, Alu.add, Alu.max)
                nc.vector.tensor_scalar_min(l1[:, h, :], l1[:, h, :], 1.0)
                # l1[f' = 256 + h*128 .. ]: w - t
                nc.vector.tensor_tensor(l1[:, 2 + h, :], w_sb[:], t[:], Alu.subtract)
                nc.vector.tensor_scalar(l1[:, 2 + h, :], l1[:, 2 + h, :], bias, 0.0, Alu.add, Alu.max)
                nc.vector.tensor_scalar_min(l1[:, 2 + h, :], l1[:, 2 + h, :], 1.0)

            # ---- MLP (exact f32; transposed orientation) ----
            l2ps = psum.tile([32, BL], f32, tag="l2ps")
            for g in range(4):
                nc.tensor.matmul(l2ps, w1_t[:, g, :], l1[:, g, :], start=(g == 0), stop=(g == 3))
            l2 = epi.tile([32, BL], f32, tag="l2")
            nc.vector.tensor_scalar(l2[:], l2ps[:], b1_t[:], 0.0, Alu.add, Alu.max)
            nc.vector.tensor_scalar_min(l2[:], l2[:], 1.0)

            l3ps = psum.tile([32, BL], f32, tag="l3ps")
            nc.tensor.matmul(l3ps, w2_t[:], l2[:], start=True, stop=True)
            l3 = epi.tile([32, BL], f32, tag="l3")
            nc.vector.tensor_scalar(l3[:], l3ps[:], b2_t[:], 0.0, Alu.add, Alu.max)
            nc.vector.tensor_scalar_min(l3[:], l3[:], 1.0)

            yps = psum.tile([1, BL], f32, tag="yps")
            nc.tensor.matmul(yps, w3_t[:], l3[:], start=True, stop=True)
            y_t = epi.tile([1, BL], f32, tag="y")
            nc.vector.tensor_scalar_add(y_t[:], yps[:], b3_t[:])
            nc.sync.dma_start(out=y, in_=y_t[:])

    nc.compile()
    return nc


def _prep_inputs(white_features, black_features, stm, W0, b0, W1, b1, W2, b2, W3, b3):
    """Host-side (lossless) relayout + batch sharding."""
    wf = np.asarray(white_features, dtype=np.float32)
    bf = np.asarray(black_features, dtype=np.float32)
    stm = np.asarray(stm, dtype=np.float32).reshape(B)
    W0 = np.asarray(W0, dtype=np.float32)
    b0 = np.asarray(b0, dtype=np.float32)
    W1 = np.asarray(W1, dtype=np.float32)
    b1 = np.asarray(b1, dtype=np.float32)
    W2 = np.asarray(W2, dtype=np.float32)
    b2 = np.asarray(b2, dtype=np.float32)
    W3 = np.asarray(W3, dtype=np.float32)
    b3 = np.asarray(b3, dtype=np.float32)

    w0_t = np.zeros((FP, M), np.float32)
    w0_t[:F] = W0.T
    b0_r = np.ascontiguousarray(b0.reshape(2, P).T)  # [128, 2]
    w1_t = np.ascontiguousarray(W1.T)  # [512, 32]
    b1_r = b1.reshape(32, 1)
    w2_t = np.ascontiguousarray(W2.T)  # [32, 32]
    b2_r = b2.reshape(32, 1)
    w3_t = np.ascontiguousarray(W3.T)  # [32, 1]
    b3_r = b3.reshape(1, 1)

    in_maps = []
    for c in range(NCORES):
        rows = slice(c * BL, (c + 1) * BL)
        xw_t = np.zeros((FP, BL), np.float32)
        xw_t[:F] = wf[rows].T
        xb_t = np.zeros((FP, BL), np.float32)
        xb_t[:F] = bf[rows].T
        stm_bc = np.ascontiguousarray(
            np.broadcast_to(stm[rows][None, :], (P, BL)), dtype=np.float32)
        in_maps.append({
            "xw_t": xw_t, "xb_t": xb_t, "w0_t": w0_t, "stm_bc": stm_bc,
            "b0_r": b0_r, "w1_t": w1_t, "b1_r": b1_r, "w2_t": w2_t,
            "b2_r": b2_r, "w3_t": w3_t, "b3_r": b3_r,
        })
    return in_maps


def _run(in_maps, trace=False, **kw):
    from concourse.bass_utils import run_bass_kernel_spmd

    if "nc" not in _cache:
        _cache["nc"] = _build_program()
    return run_bass_kernel_spmd(
        _cache["nc"], in_maps, core_ids=list(range(NCORES)), trace=trace, **kw)


def kernel(**inputs) -> np.ndarray:
    in_maps = _prep_inputs(**inputs)
    res = _run(in_maps, trace=False)
    out = np.empty((B, 1), np.float32)
    for c in range(NCORES):
        out[c * BL : (c + 1) * BL, 0] = res.results[c]["y"].reshape(BL)
    return out


# revision 9
# speedup vs baseline: 1.2102x; 1.0106x over previous
"""Trainium2 Bass kernel for the ChessNN (NNUE-style) model.

Computation:
    w   = white @ W0.T + b0                  # [B, 256]
    b   = black @ W0.T + b0                  # [B, 256]
    acc = stm * [w, b] + (1-stm) * [b, w]    # [B, 512]
    l1  = clip(acc, 0, 1)
    l2  = clip(l1 @ W1.T + b1, 0, 1)         # [B, 32]
    l3  = clip(l2 @ W2.T + b2, 0, 1)         # [B, 32]
    y   = l3 @ W3.T + b3                     # [B, 1]

Strategy: data-parallel over the batch across 8 NeuronCores (512 rows each).
The l0 GEMM streams X and W0 feature-major (host-transposed, lossless) so the
PE contracts over the partition dim with no on-chip transposes; inputs are
cast f32->bf16 during the SWDGE DMA (f32 accumulation in PSUM), and the whole
epilogue runs in the transposed orientation [features, batch] where every bias
is a per-partition scalar and the final [1, 512] result stores contiguously.
"""

import numpy as np

NCORES = 8
B = 4096
BL = B // NCORES  # 512 batch rows per core
F = 41024
P = 128
KT = 321  # ceil(F / P) k-tiles
FP = KT * P  # 41088, feature dim zero-padded
M = 256  # l0 output width
GROUP = 4  # steady-state k-tiles per DMA chunk (small: keeps PE gaps under the ~3.4us HAM re-throttle window)
RAMP = [2, 2]  # small leading chunks so the PE starts early

# bf16 multiplies with f32 accumulation for the big GEMM (4x faster PE than
# fp32's 2-pass mode); the tiny MLP stays exact f32.
GEMM_BF16 = True

_cache = {}


def _build_program():
    import concourse.mybir as mybir
    from concourse import bacc
    from concourse.tile import TileContext

    f32 = mybir.dt.float32
    gdt = mybir.dt.bfloat16 if GEMM_BF16 else f32
    Alu = mybir.AluOpType

    nc = bacc.Bacc("TRN2", target_bir_lowering=False, debug=False,
                   enable_asserts=False)

    xw = nc.dram_tensor("xw_t", [FP, BL], f32, kind="ExternalInput").ap()
    xb = nc.dram_tensor("xb_t", [FP, BL], f32, kind="ExternalInput").ap()
    w0 = nc.dram_tensor("w0_t", [FP, M], f32, kind="ExternalInput").ap()
    stm = nc.dram_tensor("stm_bc", [P, BL], f32, kind="ExternalInput").ap()
    b0 = nc.dram_tensor("b0_r", [P, 2], f32, kind="ExternalInput").ap()
    w1 = nc.dram_tensor("w1_t", [2 * M, 32], f32, kind="ExternalInput").ap()
    b1 = nc.dram_tensor("b1_r", [32, 1], f32, kind="ExternalInput").ap()
    w2 = nc.dram_tensor("w2_t", [32, 32], f32, kind="ExternalInput").ap()
    b2 = nc.dram_tensor("b2_r", [32, 1], f32, kind="ExternalInput").ap()
    w3 = nc.dram_tensor("w3_t", [32, 1], f32, kind="ExternalInput").ap()
    b3 = nc.dram_tensor("b3_r", [1, 1], f32, kind="ExternalInput").ap()
    y = nc.dram_tensor("y", [1, BL], f32, kind="ExternalOutput").ap()

    with TileContext(nc) as tc:
        with (
            tc.tile_pool(name="xpool", bufs=10) as xpool,
            tc.tile_pool(name="wpool", bufs=10) as wpool,
            tc.tile_pool(name="cpool", bufs=1) as cpool,
            tc.tile_pool(name="epi", bufs=1) as epi,
            tc.tile_pool(name="psum", bufs=1, space="PSUM") as psum,
        ):
            # ---- constants (loaded once) ----
            stm_t = cpool.tile([P, BL], f32, tag="stm")
            nc.sync.dma_start(out=stm_t[:], in_=stm)
            b0_t = cpool.tile([P, 2], f32, tag="b0")
            nc.sync.dma_start(out=b0_t[:], in_=b0)
            w1_t = cpool.tile([P, 4, 32], f32, tag="w1")
            nc.sync.dma_start(out=w1_t[:], in_=w1.rearrange("(g p) m -> p g m", p=P))
            b1_t = cpool.tile([32, 1], f32, tag="b1")
            nc.sync.dma_start(out=b1_t[:], in_=b1)
            w2_t = cpool.tile([32, 32], f32, tag="w2")
            nc.sync.dma_start(out=w2_t[:], in_=w2)
            b2_t = cpool.tile([32, 1], f32, tag="b2")
            nc.sync.dma_start(out=b2_t[:], in_=b2)
            w3_t = cpool.tile([32, 1], f32, tag="w3")
            nc.sync.dma_start(out=w3_t[:], in_=w3)
            b3_t = cpool.tile([1, 1], f32, tag="b3")
            nc.sync.dma_start(out=b3_t[:], in_=b3)

            # ---- l0 GEMM: accT[f', b] accumulated over 321 k-tiles ----
            # acc[0]=white h0, acc[1]=white h1, acc[2]=black h0, acc[3]=black h1
            acc = [psum.tile([P, BL], f32, tag=f"acc{i}", name=f"acc{i}") for i in range(4)]

            k0 = 0
            rest = KT - sum(RAMP)
            groups = RAMP + [GROUP] * (rest // GROUP) + ([rest % GROUP] if rest % GROUP else [])
            for gsz in groups:
                xw_t = xpool.tile([P, gsz, BL], gdt, tag="xw")
                xb_t = xpool.tile([P, gsz, BL], gdt, tag="xb")
                w0_t = wpool.tile([P, gsz, M], gdt, tag="w0")
                rows = slice(k0 * P, (k0 + gsz) * P)
                dma = nc.gpsimd if GEMM_BF16 else nc.sync
                dma.dma_start(out=w0_t[:], in_=w0[rows, :].rearrange("(g p) m -> p g m", p=P))
                dma.dma_start(out=xw_t[:], in_=xw[rows, :].rearrange("(g p) b -> p g b", p=P))
                dma.dma_start(out=xb_t[:], in_=xb[rows, :].rearrange("(g p) b -> p g b", p=P))
                for g in range(gsz):
                    k = k0 + g
                    st, sp = (k == 0), (k == KT - 1)
                    # consecutive matmuls share the stationary operand
                    nc.tensor.matmul(acc[0], w0_t[:, g, 0:P], xw_t[:, g, :], start=st, stop=sp)
                    nc.tensor.matmul(acc[2], w0_t[:, g, 0:P], xb_t[:, g, :], start=st, stop=sp)
                    nc.tensor.matmul(acc[1], w0_t[:, g, P:M], xw_t[:, g, :], start=st, stop=sp)
                    nc.tensor.matmul(acc[3], w0_t[:, g, P:M], xb_t[:, g, :], start=st, stop=sp)
                k0 += gsz

            # ---- blend + bias + clip -> l1T [512 f', 512 b] ----
            # acc col f' < 256:  stm*w + (1-stm)*b = b + stm*(w-b)
            # acc col f' >= 256: stm*b + (1-stm)*w = w - stm*(w-b)
            l1 = epi.tile([P, 4, BL], f32, tag="l1")
            for h in range(2):
                w_ps, b_ps = acc[h], acc[2 + h]
                # HW: an op may read at most one non-scalar input from PSUM.
                w_sb = epi.tile([P, BL], f32, tag=f"wsb{h}")
                nc.vector.tensor_copy(out=w_sb[:], in_=w_ps[:])
                t = epi.tile([P, BL], f32, tag=f"t{h}")
                nc.vector.tensor_tensor(t[:], w_sb[:], b_ps[:], Alu.subtract)
                nc.vector.tensor_tensor(t[:], t[:], stm_t[:], Alu.mult)
                bias = b0_t[:, h : h + 1]
                # l1[f' = h*128 .. ]: b + t
                nc.vector.tensor_tensor(l1[:, h, :], b_ps[:], t[:], Alu.add)
                nc.vector.tensor_scalar(l1[:, h, :], l1[:, h, :], bias, 0.0, Alu.add, Alu.max)
                nc.vector.tensor_scalar_min(l1[:, h, :], l1[:, h, :], 1.0)
                # l1[f' = 256 + h*128 .. ]: w - t
                nc.vector.tensor_tensor(l1[:, 2 + h, :], w_sb[:], t[:], Alu.subtract)
                nc.vector.tensor_scalar(l1[:, 2 + h, :], l1[:, 2 + h, :], bias, 0.0, Alu.add, Alu.max)
                nc.vector.tensor_scalar_min(l1[:, 2 + h, :], l1[:, 2 + h, :], 1.0)

            # ---- MLP (exact f32; transposed orientation) ----
            l2ps = psum.tile([32, BL], f32, tag="l2ps")
            for g in range(4):
                nc.tensor.matmul(l2ps, w1_t[:, g, :], l1[:, g, :], start=(g == 0), stop=(g == 3))
            l2 = epi.tile([32, BL], f32, tag="l2")
            nc.vector.tensor_scalar(l2[:], l2ps[:], b1_t[:], 0.0, Alu.add, Alu.max)
            nc.vector.tensor_scalar_min(l2[:], l2[:], 1.0)

            l3ps = psum.tile([32, BL], f32, tag="l3ps")
            nc.tensor.matmul(l3ps, w2_t[:], l2[:], start=True, stop=True)
            l3 = epi.tile([32, BL], f32, tag="l3")
            nc.vector.tensor_scalar(l3[:], l3ps[:], b2_t[:], 0.0, Alu.add, Alu.max)
            nc.vector.tensor_scalar_min(l3[:], l3[:], 1.0)

            yps = psum.tile([1, BL], f32, tag="yps")
            nc.tensor.matmul(yps, w3_t[:], l3[:], start=True, stop=True)
            y_t = epi.tile([1, BL], f32, tag="y")
            nc.vector.tensor_scalar_add(y_t[:], yps[:], b3_t[:])
            nc.sync.dma_start(out=y, in_=y_t[:])

    nc.compile()
    return nc


def _prep_inputs(white_features, black_features, stm, W0, b0, W1, b1, W2, b2, W3, b3):
    """Host-side (lossless) relayout + batch sharding."""
    wf = np.asarray(white_features, dtype=np.float32)
    bf = np.asarray(black_features, dtype=np.float32)
    stm = np.asarray(stm, dtype=np.float32).reshape(B)
    W0 = np.asarray(W0, dtype=np.float32)
    b0 = np.asarray(b0, dtype=np.float32)
    W1 = np.asarray(W1, dtype=np.float32)
    b1 = np.asarray(b1, dtype=np.float32)
    W2 = np.asarray(W2, dtype=np.float32)
    b2 = np.asarray(b2, dtype=np.float32)
    W3 = np.asarray(W3, dtype=np.float32)
    b3 = np.asarray(b3, dtype=np.float32)

    w0_t = np.zeros((FP, M), np.float32)
    w0_t[:F] = W0.T
    b0_r = np.ascontiguousarray(b0.reshape(2, P).T)  # [128, 2]
    w1_t = np.ascontiguousarray(W1.T)  # [512, 32]
    b1_r = b1.reshape(32, 1)
    w2_t = np.ascontiguousarray(W2.T)  # [32, 32]
    b2_r = b2.reshape(32, 1)
    w3_t = np.ascontiguousarray(W3.T)  # [32, 1]
    b3_r = b3.reshape(1, 1)

    in_maps = []
    for c in range(NCORES):
        rows = slice(c * BL, (c + 1) * BL)
        xw_t = np.zeros((FP, BL), np.float32)
        xw_t[:F] = wf[rows].T
        xb_t = np.zeros((FP, BL), np.float32)
        xb_t[:F] = bf[rows].T
        stm_bc = np.ascontiguousarray(
            np.broadcast_to(stm[rows][None, :], (P, BL)), dtype=np.float32)
        in_maps.append({
            "xw_t": xw_t, "xb_t": xb_t, "w0_t": w0_t, "stm_bc": stm_bc,
            "b0_r": b0_r, "w1_t": w1_t, "b1_r": b1_r, "w2_t": w2_t,
            "b2_r": b2_r, "w3_t": w3_t, "b3_r": b3_r,
        })
    return in_maps


def _run(in_maps, trace=False, **kw):
    from concourse.bass_utils import run_bass_kernel_spmd

    if "nc" not in _cache:
        _cache["nc"] = _build_program()
    return run_bass_kernel_spmd(
        _cache["nc"], in_maps, core_ids=list(range(NCORES)), trace=trace, **kw)


def kernel(**inputs) -> np.ndarray:
    in_maps = _prep_inputs(**inputs)
    res = _run(in_maps, trace=False)
    out = np.empty((B, 1), np.float32)
    for c in range(NCORES):
        out[c * BL : (c + 1) * BL, 0] = res.results[c]["y"].reshape(BL)
    return out
